# revision 21
# baseline (speedup 1.0000x reference)
"""Trainium2 Bass kernel for AttributeAttentionModule.

y = attention over heads of QKV projections:
  Q = sa @ Wq.T + bq ; K = x @ Wk.T + bk ; V = x @ Wv.T + bv   (all [B, D])
  per-sample scores[h,g] = Q_h . K_g / 32 ; softmax over g ; out_h = sum_g w_hg V_g

Data-parallel over 8 NeuronCores (batch sharded). Q/K projections run in
fp8 e4m3 with DoubleRow perf mode (157 TF/s: 2 MACs/PE/cycle, 256-deep
contraction per instruction); V runs in bf16 (fp8 noise in V would pass
straight through the softmax-convex combination at unit gain, while Q/K
noise is strongly damped by the softmax). Weights are prescaled by 128 for
fp8 (raw weights sit below e4m3's normal range); the 1/128^2 is folded into
the softmax exp scale. Attention is fully fused in SBUF with no DRAM
round-trip: Q persists per group of 4 batch-tiles, score partials are
accumulated inline (accum_out dots) as each K PSUM chunk lands so K is
never stored, and each V PSUM chunk is immediately folded into per-head
f32 output accumulators which DMA out as soon as their last head chunk
lands. PSUM is double-buffered (4 tags x 2 bufs = all 8 banks); weight
tiles are deep-buffered and the next group's first Q-weight tiles prefetch
during the V pass; the last V sweep runs batch-tile-major so the final
combine overlaps the last matmuls. Measured ~1.02 ms vs the 0.995 ms
precision-constrained roofline (Q/K fp8 + V bf16) on 8 trn2 cores.
"""

import os
import sys

for _p in ("/opt/trn_rl_repo", "/root/.axon_site/_ro/trn_rl_repo"):
    if os.path.isdir(_p) and _p not in sys.path:
        sys.path.append(_p)

import numpy as np
import ml_dtypes
from contextlib import ExitStack

B = 16384
D = 3072
H = 3
DH = D // H          # 1024
NCORES = 8
P = 128              # partition tile
NO = 512             # matmul moving free dim (one PSUM bank of fp32)
NOT = D // NO        # 6 output-column tiles
KT = D // P          # 24 contraction tiles of 128
K8 = KT // 2         # 12 fp8 DoubleRow contraction tiles of 256
WS = 128.0           # fp8 weight prescale (power of two, exact)
ESCALE = 1.0 / (32.0 * WS * WS)  # softmax exp scale: 1/sqrt(dh) / WS^2
GBT = 4              # batch tiles per weight-streaming group

E4 = ml_dtypes.float8_e4m3
BF = ml_dtypes.bfloat16

_CACHE = {}


def _build(bs=B // NCORES, gbt=GBT):
    import concourse.bass as bass
    import concourse.tile as tile
    from concourse import bacc, mybir

    f32 = mybir.dt.float32
    f8 = mybir.dt.float8e4
    bf16 = mybir.dt.bfloat16
    mult = mybir.AluOpType.mult
    add = mybir.AluOpType.add
    bypass = mybir.AluOpType.bypass
    Exp = mybir.ActivationFunctionType.Exp
    DR = mybir.MatmulPerfMode.DoubleRow

    nbt = bs // P        # batch tiles per core
    ng = nbt // gbt      # weight-stream groups

    nc = bacc.Bacc(
        "TRN2", target_bir_lowering=False, debug=False, num_devices=NCORES
    )

    # pre-tiled inputs (see kernel() for host layouts)
    sa8d = nc.dram_tensor("sa8", [nbt, P, K8, 2, P], f8, kind="ExternalInput").ap()
    x8d = nc.dram_tensor("x8", [nbt, P, K8, 2, P], f8, kind="ExternalInput").ap()
    x16d = nc.dram_tensor("x16", [nbt, P, KT, P], bf16, kind="ExternalInput").ap()
    wq8d = nc.dram_tensor("wq8", [NOT, 4, P, 3, 2, NO], f8, kind="ExternalInput").ap()
    wk8d = nc.dram_tensor("wk8", [NOT, 4, P, 3, 2, NO], f8, kind="ExternalInput").ap()
    wv16d = nc.dram_tensor("wv16", [NOT, 8, P, 3, NO], bf16, kind="ExternalInput").ap()
    bqd = nc.dram_tensor("bq128", [P, D], bf16, kind="ExternalInput").ap()
    bkd = nc.dram_tensor("bk128", [P, D], bf16, kind="ExternalInput").ap()
    bvd = nc.dram_tensor("bv", [P, D], bf16, kind="ExternalInput").ap()
    outd = nc.dram_tensor("out", [bs, D], f32, kind="ExternalOutput").ap()

    with tile.TileContext(nc) as tc, ExitStack() as ctx:
        sapool = ctx.enter_context(tc.tile_pool(name="sapool", bufs=1))
        x8pool = ctx.enter_context(tc.tile_pool(name="x8pool", bufs=1))
        x16pool = ctx.enter_context(tc.tile_pool(name="x16pool", bufs=1))
        qpool = ctx.enter_context(tc.tile_pool(name="qpool", bufs=1))
        accpool = ctx.enter_context(tc.tile_pool(name="accpool", bufs=1))
        wpool = ctx.enter_context(tc.tile_pool(name="wpool", bufs=6))
        wvpool = ctx.enter_context(tc.tile_pool(name="wvpool", bufs=8))
        bpool = ctx.enter_context(tc.tile_pool(name="bpool", bufs=1))
        pspool = ctx.enter_context(tc.tile_pool(name="psum", bufs=2, space="PSUM"))
        kocpool = ctx.enter_context(tc.tile_pool(name="kocp", bufs=4))
        prodpool = ctx.enter_context(tc.tile_pool(name="prodp", bufs=2))
        smallp = ctx.enter_context(tc.tile_pool(name="smallp", bufs=4))

        bias_loaded = False
        bias_t = {}
        pre_wq = None  # next group's o=0 Q-weight tiles, prefetched in V pass

        for g in range(ng):
            bts = [g * gbt + i for i in range(gbt)]

            # activation loads for this group (sync queue: independent of
            # the weight stream on gpsimd so they land during prior passes).
            # Group 0's x8/x16 loads are deferred into the Q pass (issued on
            # the gpsimd stream) so the head isn't one giant DMA burst.
            saT, x8T, x16T = [], [], []
            for i, bt in enumerate(bts):
                t = sapool.tile([P, K8, 2, P], f8, tag=f"sa{i}", name=f"sa{i}")
                nc.sync.dma_start(t[:], sa8d[bt])
                saT.append(t)
            if not bias_loaded:
                # after the first group's Q stationaries so they don't delay
                # the first matmul; they are only needed at the first q_sink
                bias_loaded = True
                for nm, src in (("q", bqd), ("k", bkd), ("v", bvd)):
                    t = bpool.tile([P, D], bf16, tag=f"b{nm}", name=f"b{nm}")
                    nc.sync.dma_start(t[:], src[:])
                    bias_t[nm] = t
            for i, bt in enumerate(bts):
                t = x8pool.tile([P, K8, 2, P], f8, tag=f"x8{i}", name=f"x8{i}")
                if g > 0:
                    nc.sync.dma_start(t[:], x8d[bt])
                x8T.append(t)
            for i, bt in enumerate(bts):
                t = x16pool.tile([P, KT, P], bf16, tag=f"x16{i}", name=f"x16{i}")
                if g > 0:
                    nc.sync.dma_start(t[:], x16d[bt])
                x16T.append(t)

            qt = [
                qpool.tile([P, D], bf16, tag=f"q{i}", name=f"q{i}")
                for i in range(gbt)
            ]
            acc = [
                accpool.tile([P, D], f32, tag=f"acc{i}", name=f"acc{i}")
                for i in range(gbt)
            ]
            s2 = [
                smallp.tile([P, 18], f32, tag=f"s2_{i}", name=f"s2_{i}")
                for i in range(gbt)
            ]
            en = [
                smallp.tile([P, 9], f32, tag=f"en_{i}", name=f"en_{i}")
                for i in range(gbt)
            ]

            def fp8_pass(wTd, acts, sink, pre=None, post_sweep=None):
                """One fp8 DoubleRow projection sweep; sink(i, o, ps) consumes
                each finished PSUM chunk. pre = prefetched o=0 weight tiles."""
                for o in range(NOT):
                    ps = [
                        pspool.tile([P, NO], f32, tag=f"ps{i}", name=f"ps{i}")
                        for i in range(gbt)
                    ]
                    for kg in range(4):
                        if o == 0 and pre is not None:
                            wt = pre[kg]
                        else:
                            wt = wpool.tile([P, 3, 2, NO], f8, tag="w8", name="w8")
                            nc.gpsimd.dma_start(wt[:], wTd[o, kg])
                        for j in range(3):
                            k8 = kg * 3 + j
                            for i in range(gbt):
                                nc.tensor.matmul(
                                    ps[i][:],
                                    acts[i][:, k8],
                                    wt[:, j],
                                    start=(k8 == 0),
                                    stop=(k8 == K8 - 1),
                                    perf_mode=DR,
                                )
                    for i in range(gbt):
                        sink(i, o, ps[i])
                    if post_sweep is not None:
                        post_sweep(o)

            # ---- Q pass: oc = ps + 128*bq -> qt (bf16) ----
            # (PSUM-draining copies run on gpsimd so PSUM recycling never
            # waits behind attention work in the vector queue)
            def q_sink(i, o, ps):
                nc.vector.tensor_add(
                    qt[i][:, o * NO : (o + 1) * NO],
                    ps[:],
                    bias_t["q"][:, o * NO : (o + 1) * NO],
                )

            def q_post_sweep(o):
                # group 0 only: stagger the K/V activation loads into the
                # Q pass instead of bursting everything at t=0
                if o == 1:
                    for i, bt in enumerate(bts):
                        nc.gpsimd.dma_start(x8T[i][:], x8d[bt])
                elif o == 3:
                    for i, bt in enumerate(bts):
                        nc.gpsimd.dma_start(x16T[i][:], x16d[bt])

            fp8_pass(
                wq8d, saT, q_sink, pre=pre_wq,
                post_sweep=(q_post_sweep if g == 0 else None),
            )
            pre_wq = None

            # ---- K pass: inline score partials, K never stored ----
            def k_sink(i, o, ps):
                g2, c = divmod(o, 2)
                koc = kocpool.tile([P, NO], bf16, tag="koc", name="koc")
                nc.vector.tensor_add(
                    koc[:], ps[:], bias_t["k"][:, o * NO : (o + 1) * NO]
                )
                for h in range(H):
                    prod = prodpool.tile([P, NO], bf16, tag="prod", name="prod")
                    nc.vector.scalar_tensor_tensor(
                        prod[:],
                        qt[i][:, h * DH + c * NO : h * DH + (c + 1) * NO],
                        1.0,
                        koc[:],
                        op0=bypass,
                        op1=mult,
                        accum_out=s2[i][:, (h * H + g2) * 2 + c : (h * H + g2) * 2 + c + 1],
                    )

            fp8_pass(wk8d, x8T, k_sink)

            # ---- softmax (tiny) -> normalized weights en ----
            for i in range(gbt):
                s = smallp.tile([P, 9], f32, tag="s", name="s")
                nc.vector.tensor_reduce(
                    s[:],
                    s2[i][:].rearrange("p (hg two) -> p hg two", two=2),
                    axis=mybir.AxisListType.X,
                    op=add,
                )
                e = smallp.tile([P, 9], f32, tag="e", name="e")
                nc.scalar.activation(e[:], s[:], Exp, scale=ESCALE)
                esum = smallp.tile([P, H], f32, tag="esum", name="esum")
                nc.vector.tensor_reduce(
                    esum[:],
                    e[:].rearrange("p (h g) -> p h g", h=H),
                    axis=mybir.AxisListType.X,
                    op=add,
                )
                rcp = smallp.tile([P, H], f32, tag="rcp", name="rcp")
                nc.vector.reciprocal(rcp[:], esum[:])
                for h in range(H):
                    nc.scalar.mul(
                        en[i][:, h * H : (h + 1) * H],
                        e[:, h * H : (h + 1) * H],
                        rcp[:, h : h + 1],
                    )

            # ---- V pass (bf16): each PSUM chunk (head g2, col-half c) is
            # combined immediately into the per-head output accumulators;
            # after the g2==2 chunk the (h, c) slices are final -> DMA out.
            for o in range(NOT):
                g2, c = divmod(o, 2)
                last_sweep = (g == ng - 1) and (o == NOT - 1)
                ps = [
                    pspool.tile([P, NO], f32, tag=f"ps{i}", name=f"ps{i}")
                    for i in range(gbt)
                ]
                wts = []
                for kg in range(8):
                    wt = wvpool.tile([P, 3, NO], bf16, tag="wv", name="wv")
                    nc.gpsimd.dma_start(wt[:], wv16d[o, kg])
                    wts.append(wt)
                    if not last_sweep:
                        for j in range(3):
                            k = kg * 3 + j
                            for i in range(gbt):
                                nc.tensor.matmul(
                                    ps[i][:],
                                    x16T[i][:, k],
                                    wt[:, j],
                                    start=(k == 0),
                                    stop=(k == KT - 1),
                                )
                if last_sweep:
                    # i-major: finish each batch tile's contraction early so
                    # its combine overlaps the remaining tiles' matmuls and
                    # only the last tile's chain runs after the final matmul
                    for i in range(gbt):
                        for kg in range(8):
                            for j in range(3):
                                k = kg * 3 + j
                                nc.tensor.matmul(
                                    ps[i][:],
                                    x16T[i][:, k],
                                    wts[kg][:, j],
                                    start=(k == 0),
                                    stop=(k == KT - 1),
                                )
                # prefetch next group's first Q-weight tiles once the V weight
                # stream is fully queued, so the V->Q boundary doesn't stall
                if o == 0 and g < ng - 1:
                    pre_wq = []
                    for kg in range(4):
                        wt = wpool.tile([P, 3, 2, NO], f8, tag="w8", name="w8")
                        nc.gpsimd.dma_start(wt[:], wq8d[0, kg])
                        pre_wq.append(wt)
                for i, bt in enumerate(bts):
                    voc = kocpool.tile([P, NO], bf16, tag="koc", name="koc")
                    nc.vector.tensor_add(
                        voc[:], ps[i][:], bias_t["v"][:, o * NO : (o + 1) * NO]
                    )
                    for h in range(H):
                        asl = acc[i][:, h * DH + c * NO : h * DH + (c + 1) * NO]
                        if g2 == 0:
                            nc.scalar.mul(asl, voc[:], en[i][:, h * H : h * H + 1])
                        else:
                            nc.vector.scalar_tensor_tensor(
                                asl,
                                voc[:],
                                en[i][:, h * H + g2 : h * H + g2 + 1],
                                asl,
                                op0=mult,
                                op1=add,
                            )
                        if g2 == 2:
                            if g == ng - 1 and c == 1:
                                dma_eng = nc.sync
                            else:
                                dma_eng = (nc.scalar, nc.sync, nc.scalar)[h]
                            dma_eng.dma_start(
                                outd[
                                    bt * P : bt * P + P,
                                    h * DH + c * NO : h * DH + (c + 1) * NO,
                                ],
                                asl,
                            )

    nc.compile()
    return nc


def _get_nc(bs=B // NCORES, gbt=GBT):
    key = (bs, gbt)
    if key not in _CACHE:
        _CACHE[key] = _build(bs, gbt)
    return _CACHE[key]


def _prep_weights(Wq, Wk, Wv, bq, bk, bv):
    """Pre-tile weights.

    fp8 Q/K: w8[o, kg, p, j, i, n] = (WS*W.T)[((kg*3+j)*2+i)*128+p, o*512+n]
    bf16 V:  wv[o, kg, p, j, n]    =      Wv.T[(kg*3+j)*128+p,     o*512+n]
    """
    ws = {}
    for nm, W in (("q", Wq), ("k", Wk)):
        wt = (np.asarray(W, dtype=np.float32).T * np.float32(WS)).astype(E4)
        w6 = wt.reshape(4, 3, 2, P, NOT, NO).transpose(4, 0, 3, 1, 2, 5)
        ws[nm] = np.ascontiguousarray(w6)
    wtv = np.asarray(Wv, dtype=np.float32).T.astype(BF)
    wv5 = wtv.reshape(8, 3, P, NOT, NO).transpose(3, 0, 2, 1, 4)
    ws["v"] = np.ascontiguousarray(wv5)

    bb = {}
    for nm, b, sc in (("q", bq, WS), ("k", bk, WS), ("v", bv, 1.0)):
        bs_ = (np.asarray(b, dtype=np.float32) * np.float32(sc)).astype(BF)
        bb[nm] = np.ascontiguousarray(np.broadcast_to(bs_, (P, D)))
    return ws, bb


def _prep_act8(a, bs):
    """fp8 DoubleRow: a8[bt, p, k8, i, b] = a[bt*128+b, (k8*2+i)*128+p]."""
    nbt = bs // P
    a8 = a.astype(E4).reshape(nbt, P, K8, 2, P).transpose(0, 4, 2, 3, 1)
    return np.ascontiguousarray(a8)


def _prep_act16(a, bs):
    """bf16: a16[bt, p, k, b] = a[bt*128+b, k*128+p]."""
    nbt = bs // P
    a16 = a.astype(BF).reshape(nbt, P, KT, P).transpose(0, 3, 2, 1)
    return np.ascontiguousarray(a16)


def _in_maps(x, sa, ws, bb, bs):
    maps = []
    for c in range(NCORES):
        r0 = c * bs
        maps.append(
            {
                "sa8": _prep_act8(sa[r0 : r0 + bs], bs),
                "x8": _prep_act8(x[r0 : r0 + bs], bs),
                "x16": _prep_act16(x[r0 : r0 + bs], bs),
                "wq8": ws["q"],
                "wk8": ws["k"],
                "wv16": ws["v"],
                "bq128": bb["q"],
                "bk128": bb["k"],
                "bv": bb["v"],
            }
        )
    return maps


def kernel(x, synthetic_attributes, Wq, bq, Wk, bk, Wv, bv, **_ignored):
    from concourse import bass_utils

    x = np.asarray(x, dtype=np.float32)
    sa = np.asarray(synthetic_attributes, dtype=np.float32)
    bs = x.shape[0] // NCORES

    ws, bb = _prep_weights(Wq, Wk, Wv, bq, bk, bv)
    nc = _get_nc(bs=bs)
    in_maps = _in_maps(x, sa, ws, bb, bs)

    res = bass_utils.run_bass_kernel_spmd(nc, in_maps, core_ids=list(range(NCORES)))
    out = np.concatenate([res.results[c]["out"] for c in range(NCORES)], axis=0)
    return out
